# revision 23
# baseline (speedup 1.0000x reference)
"""Trainium2 Bass kernel for Bert_BiLSTM_CRF loss.

Model (per reference): 2-layer BiLSTM (E=768 -> 2x128, then 256 -> 2x128),
linear to K=11 emissions, CRF loss = -mean(num - den) with all-ones mask.

Sharding: pure data parallel, batch 64 -> 8 cores x 8 examples.

Device layout choices (per core, BL=8 examples):
 - Everything transposed: LSTM state h^T is [H=128 partitions, BL free];
   gates are [4H split into 4 chunks of 128 partitions, BL free]. Elementwise
   work thus uses all 128 partitions.
 - Input projections W_ih @ x for all t are dense matmuls (throughput-bound),
   written into SBUF "slabs" of 64 timesteps, fused with the recurrence
   (no DRAM round trip). Gate order host-reordered to (f,i,o,g) so a single
   sigmoid covers 24 contiguous columns and tanh the last 8.
 - Recurrence per step: 4x (LDW+matmul) for W_hh @ h, DVE add of the
   precomputed projection, ACT sigmoid/tanh, DVE cell/output updates.
 - Emissions em^T = W_lin @ h1^T + b in [K=11 partitions, T*BL free].
 - CRF forward pass runs in the exp domain: P <- (expTrans/K)^T @ P * exp(em_t)
   (one tiny matmul + one DVE multiply per step), renormalized every RENORM
   steps by the column sum (log accumulated). Host finishes with
   den = log(sum_j P_j * exp(end_j)) + logz + (T-1)*log(K).
 - The numerator is pure gathers over (tags, trans, em); em is shipped back
   (180KB/core) and the O(B*T) scalar assembly happens on host in fp64.

The mask input is all ones (per the problem spec fill) and is treated as such.
"""

import numpy as np
from contextlib import ExitStack

B, T, E, HID, K = 64, 512, 768, 256, 11
H = 128            # per-direction hidden
G4 = 4 * H         # 512 gate rows
NCORES = 8
BL = B // NCORES   # 8 examples per core
NTILE = 512        # projection n-tile columns
SLAB_T = NTILE // BL  # 64 timesteps per slab
RENORM = 32

_GATE_PERM = np.concatenate([   # pytorch (i,f,g,o) rows -> (f,i,o,g)
    np.arange(H, 2 * H),        # f
    np.arange(0, H),            # i
    np.arange(3 * H, 4 * H),    # o
    np.arange(2 * H, 3 * H),    # g
])


def _build(reps: int = 1, t_len: int = T, taps: bool = False,
           phases: tuple = ("l0", "l1", "em", "crf")):
    """Builds the Bacc program. Returns (nc, meta)."""
    import concourse.bacc as bacc
    import concourse.tile as tile
    import concourse.mybir as mybir

    fp32 = mybir.dt.float32
    bf16 = mybir.dt.bfloat16
    AF = mybir.ActivationFunctionType

    NT = t_len * BL
    NSLAB = NT // NTILE
    assert t_len % SLAB_T == 0

    nc = bacc.Bacc("TRN2", target_bir_lowering=False, debug=False,
                   num_devices=NCORES)

    from concourse.alu_op_type import AluOpType as ALU

    def din(name, shape, dt=fp32):
        return nc.dram_tensor(name, shape, dt, kind="ExternalInput").ap()

    def dout(name, shape):
        return nc.dram_tensor(name, shape, fp32, kind="ExternalOutput").ap()

    xT_d = din("xT", [E, NT], bf16)
    wih0_d = {d: din(f"wih0{d}", [128, 6 * 512], bf16) for d in "fr"}
    wih1_d = {d: din(f"wih1{d}", [128, 2 * 512], bf16) for d in "fr"}
    whh_d = {(l, d): din(f"whh{l}{d}", [128, 512], bf16)
             for l in (0, 1) for d in "fr"}
    bias_d = {(l, d): din(f"biasrow{l}{d}", [1, 512])
              for l in (0, 1) for d in "fr"}
    wlin_d = din("wlinT", [128, 2 * K], bf16)
    linb_d = din("linb", [K, 1])
    expT_d = din("expT", [K, K])
    expstart_d = din("expstart", [K, 1])

    emT_o = dout("emT", [K, NT])
    pfin_o = dout("pfin", [K, BL])
    logz_o = dout("logz", [1, BL])
    taps_o = {}
    if taps:
        for nm in ("h0f", "h0r", "h1f", "h1r"):
            taps_o[nm] = dout(nm, [128, NT])

    with tile.TileContext(nc) as tc, ExitStack() as ctx:
        wp = ctx.enter_context(tc.tile_pool(name="weights", bufs=1))

        def load_w(ap, shape, dt=fp32):
            t = wp.tile(shape, dt, name=f"w_{ap.tensor.name}")
            nc.sync.dma_start(t[:], ap[:, :])
            return t

        s_wih = {(0, d): load_w(wih0_d[d], [128, 6 * 512], bf16) for d in "fr"}
        s_wih.update({(1, d): load_w(wih1_d[d], [128, 2 * 512], bf16)
                      for d in "fr"})
        s_whh = {k: load_w(v, [128, 512], bf16) for k, v in whh_d.items()}
        s_bias = {k: load_w(v, [1, 512]) for k, v in bias_d.items()}
        s_wlin = load_w(wlin_d, [128, 2 * K], bf16)
        s_ones1 = wp.tile([1, 256], fp32, name="ones1")
        nc.vector.memset(s_ones1[:], 1.0)
        s_linb = load_w(linb_d, [K, 1])
        s_expT = load_w(expT_d, [K, K])
        s_expstart = load_w(expstart_d, [K, 1])
        s_ones = wp.tile([K, K], fp32, name="ones")
        nc.vector.memset(s_ones[:], 1.0)

        # h segments persist across phases within a rep
        for _rep in range(reps):
            with ExitStack() as rctx:
                hp = rctx.enter_context(tc.tile_pool(name="hsegs", bufs=1))
                h0 = {d: [hp.tile([128, NTILE], bf16, name=f"h0{d}{j}") for j in range(NSLAB)]
                      for d in "fr"}
                h1 = {d: [hp.tile([128, NTILE], bf16, name=f"h1{d}{j}") for j in range(NSLAB)]
                      for d in "fr"}

                def lstm_layer(layer, h_out):
                    """Projection + recurrence for one BiLSTM layer.

                    Projections (W_ih @ x and the bias via a ones-row
                    matmul) accumulate straight into per-step PSUM blocks
                    of 32 wall steps; the per-step W_hh @ h matmuls then
                    accumulate on top (all PSUM writers are PE, so the
                    has_written accumulate semantics hold), and the gate
                    sigmoid reads PSUM directly. This removes the DVE
                    pre-add from the recurrence critical path."""
                    BT = 32             # wall steps per PSUM block
                    NBLK = t_len // BT
                    with ExitStack() as lctx:
                        psp = lctx.enter_context(
                            tc.tile_pool(name=f"psb{layer}", bufs=2,
                                         space="PSUM"))
                        scr = lctx.enter_context(
                            tc.tile_pool(name=f"scr{layer}", bufs=2))
                        stp = lctx.enter_context(
                            tc.tile_pool(name=f"st{layer}", bufs=1))
                        xtp = None
                        if layer == 0:
                            xtp = lctx.enter_context(
                                tc.tile_pool(name="xt", bufs=2))

                        blocks = {"f": [None] * NBLK, "r": [None] * NBLK}

                        def proj(d, hs):
                            # time block covered at wall block hs
                            tblk = hs if d == "f" else (NBLK - 1 - hs)
                            c0 = tblk * BT * BL
                            # gate-major layout: col = g*256 + ls*8 + b so
                            # every matmul writes a flat contiguous slice
                            ps = psp.tile([128, BT * 32], fp32,
                                          name=f"pp{d}{hs}", tag=f"pp{d}")
                            if layer == 0:
                                xt = xtp.tile([128, 6 * BT * BL], bf16,
                                              name=f"xt{d}{hs}", tag=f"xt{d}")
                                nk = 6
                                for k in range(nk):
                                    nc.sync.dma_start(
                                        xt[:, k * 256:(k + 1) * 256],
                                        xT_d[k * 128:(k + 1) * 128,
                                             c0:c0 + BT * BL])
                                rhs = [xt[:, k * 256:(k + 1) * 256]
                                       for k in range(nk)]
                            else:
                                j, half = tblk // 2, (tblk % 2) * 256
                                rhs = [h0["f"][j][:, half:half + 256],
                                       h0["r"][j][:, half:half + 256]]
                            nk = len(rhs)
                            w = s_wih[(layer, d)]
                            br = s_bias[(layer, d)]
                            NB = BT * BL
                            for m in range(4):
                                # bias first (start=True resets has_written)
                                nc.tensor.matmul(
                                    ps[:, m * NB:(m + 1) * NB],
                                    br[:, m * 128:(m + 1) * 128],
                                    s_ones1[:, :],
                                    start=True, stop=False)
                                for k in range(nk):
                                    nc.tensor.matmul(
                                        ps[:, m * NB:(m + 1) * NB],
                                        w[:, k * 512 + m * 128:
                                          k * 512 + (m + 1) * 128],
                                        rhs[k],
                                        start=False, stop=(k == nk - 1))
                            blocks[d][hs] = ps

                        for hs in range(NBLK):
                            proj("f", hs)
                            proj("r", hs)

                        # recurrence, fwd and rev interleaved per wall step
                        S = {d: [stp.tile([128, 16], fp32, name=f"S{d}{i}") for i in (0, 1)]
                             for d in "fr"}
                        nc.vector.memset(S["f"][0][:, 0:8], 0.0)
                        nc.vector.memset(S["r"][0][:, 0:8], 0.0)

                        def step(d, s):
                            t = s if d == "f" else (t_len - 1 - s)
                            ps = blocks[d][s // BT]
                            lt = t % BT
                            NB = BT * BL
                            # strided gate view for this step: cols
                            # {g*NB + lt*8 + b}
                            pre = ps[:].rearrange(
                                "p (g ls b) -> p ls g b", b=BL,
                                g=4)[:, lt, :, :]
                            if s > 0:
                                tprev = (s - 1) if d == "f" else (t_len - s)
                                hseg = h_out[d][tprev // SLAB_T]
                                hoff = (tprev % SLAB_T) * BL
                                whh = s_whh[(layer, d)]
                                for m in range(4):
                                    nc.tensor.matmul(
                                        ps[:, m * NB + lt * BL:
                                           m * NB + (lt + 1) * BL],
                                        whh[:, m * 128:(m + 1) * 128],
                                        hseg[:, hoff:hoff + BL],
                                        start=False, stop=True,
                                        skip_group_check=True)
                            Scur = S[d][s % 2]
                            Snxt = S[d][(s + 1) % 2]
                            # gate preacts are (f,i,o,2g); one sigmoid covers
                            # all 32 cols, then tanh(g) = 2*sig(2g) - 1 on DVE
                            sig = scr.tile([128, 32], fp32, name=f"sig{d}{s}", tag=f"sig{d}")
                            nc.scalar.activation(sig[:], pre,
                                                 AF.Sigmoid)
                            nc.vector.tensor_scalar(Scur[:, 8:16],
                                                    sig[:, 24:32], 2.0, -1.0,
                                                    ALU.mult, ALU.add)
                            tmp = scr.tile([128, 16], fp32, name=f"tmp{d}{s}", tag=f"tmp{d}")
                            nc.vector.tensor_mul(tmp[:], sig[:, 0:16],
                                                 Scur[:, 0:16])
                            nc.vector.tensor_add(Snxt[:, 0:8], tmp[:, 0:8],
                                                 tmp[:, 8:16])
                            tc8 = scr.tile([128, 8], fp32, name=f"tc{d}{s}", tag=f"tc{d}")
                            nc.scalar.activation(tc8[:], Snxt[:, 0:8], AF.Tanh)
                            hseg_o = h_out[d][t // SLAB_T]
                            nc.vector.tensor_mul(
                                hseg_o[:, (t % SLAB_T) * BL:
                                       (t % SLAB_T + 1) * BL],
                                sig[:, 16:24], tc8[:])

                        for s in range(t_len):
                            step("f", s)
                            step("r", s)

                if "l0" in phases:
                    lstm_layer(0, h0)
                if "l1" in phases:
                    lstm_layer(1, h1)

                if "em" not in phases:
                    continue
                # emissions + CRF
                with ExitStack() as ectx:
                    emp = ectx.enter_context(tc.tile_pool(name="em", bufs=1))
                    em_ps = ectx.enter_context(
                        tc.tile_pool(name="emps", bufs=2, space="PSUM"))
                    crf_ps = ectx.enter_context(
                        tc.tile_pool(name="crfps", bufs=2, space="PSUM"))
                    crf_sc = ectx.enter_context(
                        tc.tile_pool(name="crfsc", bufs=2))

                    emT = emp.tile([K, NT], fp32, name="emTt")
                    expem = emp.tile([K, NT], fp32, name="expem")
                    for j in range(NSLAB):
                        ps = em_ps.tile([K, NTILE], fp32, name=f"emps{j}", tag="emps")
                        for k, d in enumerate("fr"):
                            nc.tensor.matmul(ps[:],
                                             s_wlin[:, k * K:(k + 1) * K],
                                             h1[d][j][:, :],
                                             start=(k == 0), stop=(k == 1))
                        nc.scalar.activation(
                            emT[:, j * NTILE:(j + 1) * NTILE], ps[:],
                            AF.Identity, bias=s_linb[:, 0:1])
                        nc.scalar.activation(
                            expem[:, j * NTILE:(j + 1) * NTILE], ps[:],
                            AF.Exp, bias=s_linb[:, 0:1])
                    nc.sync.dma_start(emT_o[:, :], emT[:])

                    if "crf" not in phases:
                        continue
                    P = [emp.tile([K, BL], fp32, name=f"P{i}") for i in (0, 1)]
                    logz = emp.tile([1, BL], fp32, name="logzt")
                    nc.vector.memset(logz[:], 0.0)
                    nc.vector.tensor_scalar_mul(P[0][:], expem[:, 0:BL],
                                                s_expstart[:, 0:1])
                    for t in range(1, t_len):
                        q = crf_ps.tile([K, BL], fp32, name=f"q{t}", tag="q")
                        nc.tensor.matmul(q[:], s_expT[:], P[(t - 1) % 2][:],
                                         start=True, stop=True)
                        nc.vector.tensor_mul(P[t % 2][:], q[:],
                                             expem[:, t * BL:(t + 1) * BL])
                        if t % RENORM == RENORM - 1 or t == t_len - 1:
                            sps = crf_ps.tile([K, BL], fp32, name=f"sps{t}", tag="sps")
                            nc.tensor.matmul(sps[:], s_ones[:], P[t % 2][:],
                                             start=True, stop=True)
                            rcp = crf_sc.tile([K, BL], fp32, name=f"rcp{t}", tag="rcp")
                            nc.vector.reciprocal(rcp[:], sps[:])
                            nc.vector.tensor_mul(P[t % 2][:], P[t % 2][:],
                                                 rcp[:])
                            lnt = crf_sc.tile([1, BL], fp32, name=f"ln{t}", tag="ln")
                            nc.scalar.activation(lnt[:], sps[0:1, :], AF.Ln)
                            nc.vector.tensor_add(logz[:], logz[:], lnt[:])
                    nc.sync.dma_start(pfin_o[:, :], P[(t_len - 1) % 2][:])
                    nc.sync.dma_start(logz_o[:, :], logz[:])

                    if taps:
                        for d in "fr":
                            for j in range(NSLAB):
                                nc.sync.dma_start(
                                    taps_o[f"h0{d}"][:, j * NTILE:
                                                     (j + 1) * NTILE],
                                    h0[d][j][:])
                                nc.sync.dma_start(
                                    taps_o[f"h1{d}"][:, j * NTILE:
                                                     (j + 1) * NTILE],
                                    h1[d][j][:])

    nc.compile()
    return nc


def _prep_weights(inp):
    """Host-side weight repacks (tiny). Returns dict of per-core-identical
    input arrays. Gate order (f,i,o,g); the g rows are pre-scaled by 2 so
    the device computes tanh(g) as 2*sigmoid(2g)-1 with a single sigmoid
    instruction over all 32 gate columns."""
    import ml_dtypes
    f32 = np.float32
    bf16 = ml_dtypes.bfloat16
    out = {}

    def pack_wih(wmat):  # [4H, din] -> [128, (din/128)*512] bf16
        w = wmat[_GATE_PERM].astype(f32)          # [512, din]
        w[384:] *= 2.0                            # g rows
        wT = np.ascontiguousarray(w.T)            # [din, 512]
        kk = wT.shape[0] // 128
        return np.ascontiguousarray(
            wT.reshape(kk, 128, 512).transpose(1, 0, 2)
            .reshape(128, kk * 512)).astype(bf16)

    def pack_whh(wmat):  # [512, 128] -> [128, 512] bf16
        w = wmat[_GATE_PERM].astype(f32)
        w[384:] *= 2.0                            # g rows
        return np.ascontiguousarray(w.T).astype(bf16)

    for l in (0, 1):
        for d, sfx in (("f", ""), ("r", "_r")):
            out[f"wih{l}{d}"] = pack_wih(inp[f"w_ih_l{l}{sfx}"])
            out[f"whh{l}{d}"] = pack_whh(inp[f"w_hh_l{l}{sfx}"])
            bsum = (inp[f"b_ih_l{l}{sfx}"] + inp[f"b_hh_l{l}{sfx}"])
            bsum = bsum[_GATE_PERM].astype(f32)
            bsum[384:] *= 2.0                     # g rows
            out[f"biasrow{l}{d}"] = np.ascontiguousarray(
                bsum.reshape(1, 512))

    lw = inp["linear_w"].astype(f32)              # [K, 256]
    out["wlinT"] = np.ascontiguousarray(
        lw.T.reshape(2, 128, K).transpose(1, 0, 2)
        .reshape(128, 2 * K)).astype(bf16)
    out["linb"] = np.ascontiguousarray(
        inp["linear_b"].astype(f32).reshape(K, 1))
    out["expT"] = np.ascontiguousarray(
        (np.exp(inp["trans"].astype(np.float64)) / K).astype(f32))
    out["expstart"] = np.ascontiguousarray(
        np.exp(inp["start_trans"].astype(np.float64)).astype(f32).reshape(K, 1))
    return out


def _host_finish(results, tags, trans, start_trans, end_trans, t_len=T):
    """Assemble the scalar loss from per-core device outputs (fp64 host)."""
    trans = np.asarray(trans, np.float64)
    start_trans = np.asarray(start_trans, np.float64)
    end_trans = np.asarray(end_trans, np.float64)
    total = 0.0
    for c in range(len(results)):
        emT = np.asarray(results[c]["emT"], np.float64)   # [K, T*BL]
        em = emT.reshape(K, t_len, BL)                    # [k, t, b]
        P = np.asarray(results[c]["pfin"], np.float64)    # [K, BL]
        lz = np.asarray(results[c]["logz"], np.float64)[0]  # [BL]
        den = (np.log((P * np.exp(end_trans)[:, None]).sum(0)) + lz
               + (t_len - 1) * np.log(K))
        tg = np.asarray(tags[c * BL:(c + 1) * BL])        # [BL, T]
        b_idx = np.arange(BL)
        # em[t, b, tags[b, t]]
        em_g = em[tg.T, np.arange(t_len)[:, None], b_idx[None, :]]  # [T, BL]
        num = (start_trans[tg[:, 0]]
               + em_g[0]
               + trans[tg[:, :-1], tg[:, 1:]].sum(axis=1)
               + em_g[1:].sum(axis=0)
               + end_trans[tg[:, -1]])
        total += (num - den).sum()
    return -total / (len(results) * BL)


def _make_in_maps(inp):
    import ml_dtypes
    embeds = np.asarray(inp["embeds"], np.float32)        # [64, T, E]
    shared = _prep_weights(inp)
    in_maps = []
    for c in range(NCORES):
        emb = embeds[c * BL:(c + 1) * BL]                 # [BL, T, E]
        xT = np.ascontiguousarray(
            emb.transpose(2, 1, 0).reshape(E, T * BL))    # col = t*BL + b
        m = dict(shared)
        m["xT"] = xT.astype(ml_dtypes.bfloat16)
        in_maps.append(m)
    return in_maps


_NC_CACHE = {}


def kernel(**inputs):
    from concourse.bass_utils import run_bass_kernel_spmd

    inp = {k: np.asarray(v) for k, v in inputs.items()}
    key = ("main", 1)
    if key not in _NC_CACHE:
        _NC_CACHE[key] = _build(reps=1)
    nc = _NC_CACHE[key]
    in_maps = _make_in_maps(inp)
    res = run_bass_kernel_spmd(nc, in_maps, core_ids=list(range(NCORES)))
    loss = _host_finish(res.results, inp["tags"], inp["trans"],
                        inp["start_trans"], inp["end_trans"])
    return np.float32(loss)

